# revision 2
# baseline (speedup 1.0000x reference)
"""Trainium2 Bass kernel for nn_ConstLoss_22746146800082 (covering-design fp8).

loss * N(N-1) = ||Cq - Ck||_F^2 = ||Aqq||^2 + ||Akk||^2 - 2||Aqk||^2 with the
feature-space Grams of row-normalized features (see kernel.py v1).  v1 streamed
the FULL 8 MB (xq+xk) to every core (each core owning 128 Gram rows), which is
DMA-bound at ~23 us on the ~360 GB/s per-core DMA fabric.

v2 shards the Gram by 128x128 BLOCKS instead: the 2048 columns of W=[Qn|Kn]
form 16 chunks of 128; block (a,b) = chunk_a^T chunk_b needs only chunks a,b.
A covering design assigns 7 chunks (3.5 MB) to each core such that all
C(16,2)+16 chunk pairs appear on some core.  Every core runs the SAME program
(SPMD): it contracts a fixed 20-position slot-pair PATTERN (2 loops + 18
edges) over its 7 resident chunk slots; the host maps slots->chunks per core
(free gather), dedups duplicated pairs, and applies the +-2/+1 weights.

Per-core: DMA 3.5 MB (~10.2 us), PE 20 blocks x 1024 cyc = 20480 cyc
(~8.5 us @2.4GHz), tail = per-block sum-of-squares via DVE bn_stats (2 blocks
per instr via an interleaved stride view: even/odd parity stats = the two
blocks) and Act square+accum, one [128, 48] f32 result DMA.
"""

import numpy as np

import concourse.bass as bass
import concourse.mybir as mybir
import concourse.tile as tile
from concourse.vector_clock import ScopedClock
from concourse.bass_utils import run_bass_kernel_spmd

N_CORES = 8
N = 4096
D = 1024
P = 128
NSLOT = 7
COLS = NSLOT * P          # 896 resident feature columns per core
NC = N // P               # 32 sample chunks
NDC = NC // 2             # 16 DoubleRow double-chunks
SCALE = 32.0

F32 = mybir.dt.float32
FP8 = mybir.dt.float8e4
DR = mybir.MatmulPerfMode.DoubleRow

# ---- covering design (cover_search.py): all 120 chunk pairs + 16 diags ----
EDGES = [(0, 1), (0, 2), (0, 3), (0, 5), (0, 6), (1, 2), (1, 3), (1, 4),
         (1, 5), (1, 6), (2, 3), (2, 4), (2, 5), (2, 6), (3, 4), (4, 5),
         (4, 6), (5, 6)]
LOOPS = [(0, 0), (1, 1)]
PATTERN = LOOPS + EDGES                      # 20 block positions
NBLK = len(PATTERN)
PHIS = [
    [15, 0, 9, 11, 13, 10, 1],
    [14, 13, 15, 7, 8, 2, 4],
    [1, 9, 5, 2, 11, 14, 10],
    [12, 4, 9, 11, 3, 7, 1],
    [6, 5, 3, 15, 12, 10, 13],
    [10, 7, 0, 2, 5, 8, 4],
    [8, 11, 6, 4, 7, 1, 9],
    [3, 2, 14, 8, 12, 6, 0],
]

N_ACT = 6                  # blocks 0..5 on Act (square + accum), close first
N_DVE = NBLK - N_ACT       # blocks 6..19 on DVE (7 parity-paired bn_stats)
N_PAIR = N_DVE // 2
OUTW_A = N_ACT
OUTW_D = 6 * N_PAIR

GROUP_SIZES = [2, 2, 4, 4, 4, 4, 4, 4, 2, 2]  # sample-chunks per DMA group


class _TC(tile.TileContext):
    """TileContext whose kernel-tail drain splits its semaphore waits across
    preceding sync-engine NOPs: this container's walrus build rejects a Drain
    carrying more than one sync wait ("Too many sync wait commands")."""

    def _drain_and_barrier(self, tick_clock, wait_clock):
        nc = self.nc
        probe = nc.sync.nop(nofuse=True)
        wait_clock.add_sem_waits(
            probe.ins, ScopedClock({None: tick_clock.global_clock})
        )
        waits = list(probe.ins.sync_info.on_wait or []) if probe.ins.sync_info else []
        if probe.ins.sync_info is not None:
            probe.ins.sync_info.on_wait = waits[:1]
        engines = [nc.vector, nc.scalar, nc.gpsimd, nc.tensor, nc.sync]
        for i, w in enumerate(waits[1:]):
            n2 = engines[i % len(engines)].nop(nofuse=True)
            n2.ins.sync_info = mybir.SyncInfo(on_wait=[w], on_update=[])
        nc.sync.drain()
        nc.all_engine_barrier()
        popped = nc._tile_sem_poison_stack.pop()
        assert popped is self._sem_poison
        nc.clear_and_free_semaphores(list(self.sems.allocated().values()))
        nc.all_engine_barrier()


MAX_WAITS_PER_INST = 1


def split_excess_waits(nc):
    """walrus rejects instructions carrying more than a couple of semaphore
    waits.  Hoist excess waits onto injected same-engine NOPs."""
    n = 0
    for f in nc.m.functions:
        for bb in f.blocks:
            insts = bb.instructions
            out = []
            changed = False
            for ins in insts:
                si = ins.sync_info
                waits = list(si.on_wait or []) if si is not None else []
                while len(waits) > MAX_WAITS_PER_INST:
                    take = waits[:MAX_WAITS_PER_INST]
                    waits = waits[MAX_WAITS_PER_INST:]
                    nop = mybir.InstNoOp(name=f"I-waitsplit-{n}", ins=[], outs=[])
                    n += 1
                    nop.engine = ins.engine
                    nop.sync_info = mybir.SyncInfo(on_wait=take, on_update=[])
                    out.append(nop)
                    changed = True
                if changed and si is not None:
                    si.on_wait = waits
                out.append(ins)
            if changed:
                bb.instructions = out
    return n


def tune_const_memsets(nc):
    """Drop unused const-tile memsets from the pre-barrier preamble; push the
    used ones past the entry barrier (their readers are in the kernel tail)."""
    used = set()
    for f in nc.m.functions:
        for bb in f.blocks:
            for ins in bb.instructions:
                for a in ins.ins:
                    m = getattr(a, "memref", None)
                    if m:
                        used.add(m)
    for f in nc.m.functions:
        if len(f.blocks) < 2:
            continue
        bb0, bb1 = f.blocks[0], f.blocks[1]
        keep, moved = [], []
        for ins in bb0.instructions:
            if ins.opcode == "Memset" and ins.outs:
                m = getattr(ins.outs[0], "memref", "")
                if m.startswith("const-"):
                    if m in used:
                        moved.append(ins)
                    continue
            keep.append(ins)
        if moved or len(keep) != len(bb0.instructions):
            bb0.instructions = keep
            bb1.instructions = moved + bb1.instructions


def fix_tail_sync(nc):
    """Tile emits conservative tail sync: every psum reader waits for ALL
    matmuls (tile-granular RAW), the first Act square waits on all DVE
    bn_stats (clock-compression proxy), and same-engine sem self-chains add
    ~160ns per instruction.  Rewrite the tail waits to precise PE ticks:
    psum region b's last write is matmul tick 300 + b + 1 (15 full dc rounds
    of NBLK matmuls, then the final-dc matmuls in block order)."""
    insts = [i for f in nc.m.functions for bb in f.blocks for i in bb.instructions]
    matmuls = [i for i in insts if i.opcode == "Matmult"]
    n_mm = len(matmuls)
    assert n_mm == NDC * NBLK, n_mm
    # find the PE completion-sem (the update attached to matmuls)
    pe_upd = None
    for i in matmuls:
        si = i.sync_info
        if si and si.on_update:
            for u in si.on_update:
                if u.ant_name and u.ant_name.startswith("PE"):
                    pe_upd = u
    assert pe_upd is not None

    def pe_wait(tick):
        return mybir.SyncWait(
            sync_type="semaphore",
            id=pe_upd.id,
            ant_name=pe_upd.ant_name,
            wait_mode="sem-ge-imm",
            wait_value=tick,
        )

    base = (NDC - 1) * NBLK  # matmul ticks before the final dc round
    bn = [i for i in insts if i.opcode == "BNStats"]
    assert len(bn) == N_PAIR
    for j, i in enumerate(bn):
        bmax = N_ACT + 2 * j + 1
        i.sync_info.on_wait = [pe_wait(base + bmax + 1)]
    acts = [i for i in insts if i.opcode == "Activation"]
    assert len(acts) == N_ACT
    # first Act square: precise PE tick covering all Act blocks (strictly
    # correct for the whole chain); later ones keep the Act self-chain, which
    # also serializes the shared hw accumulator register.
    acts[0].sync_info.on_wait = [pe_wait(base + N_ACT)]


def fix_prepared_out(nc):
    """Tile routes each Pool-engine DMA prep to a DMASW semaphore lane and
    the kernel-exit drain waits for those lane sems to reach 16, but the
    completion sem baked into a prepare_only descriptor is the user-passed
    `sem=` handle.  Point each prep's completion update at its assigned
    DMASW lane sem (found from the drain's own waits, in round-robin order)
    so the hardware DMA-complete increments what the drain is waiting on."""
    insts = [i for f in nc.m.functions for bb in f.blocks for i in bb.instructions]
    lane_sems = {}
    for i in insts:
        si = i.sync_info
        for w in (si.on_wait or []) if si else []:
            if w.ant_name and w.ant_name.startswith("DMASW"):
                lane = int(w.ant_name[5 : w.ant_name.index("_")])
                lane_sems[lane] = (w.id, w.ant_name)
    preps = [i for i in insts if i.opcode == "KVWritebackAnt"]
    assert len(preps) == len(lane_sems), (len(preps), lane_sems)
    for j, i in enumerate(preps):
        sid, sname = lane_sems[j]
        for u in i.sync_info.on_update or []:
            if u.ant_name == "out_dma_sem":
                u.id = sid
                u.ant_name = sname


def build_program(sim_mode: bool = False):
    nc = bass.Bass(
        "TRN2", target_bir_lowering=False, debug=False, num_devices=N_CORES
    )
    x = nc.dram_tensor("x", [N, COLS], FP8, kind="ExternalInput").ap()
    out_a = nc.dram_tensor("out_a", [P, OUTW_A], F32, kind="ExternalOutput").ap()
    out_dv = nc.dram_tensor("out_dv", [P, OUTW_D], F32, kind="ExternalOutput").ap()

    with _TC(nc) as tc:
        with (
            tc.tile_pool(name="stream", bufs=1) as stream,
            tc.tile_pool(name="tail", bufs=1) as tail,
            tc.tile_pool(name="psum", bufs=1, space="PSUM") as psum,
        ):
            ps = psum.tile([P, NBLK * P], F32, name="ps", tag="ps")

            acc_a = tail.tile([P, OUTW_A], F32, name="acc_a")
            acc_d = tail.tile([P, OUTW_D], F32, name="acc_d")

            xr = x.rearrange("(c p) d -> p c d", p=P)
            off = 0
            dc = 0
            for g, m in enumerate(GROUP_SIZES):
                tg = stream.tile([P, m, COLS], FP8, name=f"s{g}", tag=f"s{g}")
                nc.sync.dma_start(out=tg, in_=xr[:, off : off + m, :])
                for jj in range(m // 2):
                    pr = slice(2 * jj, 2 * jj + 2)
                    for pos, (u, v) in enumerate(PATTERN):
                        # start=True zeroes the ENTIRE psum bank, so only the
                        # first block of each 4-block bank may carry it; the
                        # other blocks accumulate onto the freshly-zeroed bank.
                        nc.tensor.matmul(
                            ps[:, P * pos : P * (pos + 1)],
                            lhsT=tg[:, pr, P * u : P * (u + 1)],
                            rhs=tg[:, pr, P * v : P * (v + 1)],
                            start=(dc == 0 and pos % 4 == 0),
                            stop=(dc == NDC - 1),
                            perf_mode=DR,
                        )
                    dc += 1
                off += m

            # ---- tail: per-block sum(A^2) partials --------------------------
            # Act: blocks 0..N_ACT-1 (their psum regions close first), square
            # activation with accumulator -> acc_a[:, b].
            # DVE: remaining blocks via parity-paired bn_stats: the input AP
            # interleaves two 128-wide blocks at element granularity (outer
            # dim stride 1, inner dim stride 128), so the even-parity stats
            # are the first block and the odd-parity stats the second; host
            # recovers sum(x^2) = M2 + n*mean^2 per parity.  Separate output
            # tiles per engine so the two reduce streams proceed
            # independently.
            scr = tail.tile([P, P], F32, name="scr")
            for p_ in range(N_PAIR):
                b0 = N_ACT + 2 * p_
                pv = ps[:, P * b0 : P * (b0 + 2)].rearrange(
                    "p (b i) -> p i b", b=2
                )
                # emit InstBNStats directly: the bass wrapper insists batched
                # outputs for multi-dim inputs, but walrus only accepts the
                # plain 6-per-partition form; the DVE streams the input AP in
                # order, so this interleaved view alternates the two blocks
                # and the even/odd parity stats separate them again.
                nc.vector.add_instruction(
                    mybir.InstBNStats(
                        name=nc.get_next_instruction_name(),
                        ins=[nc.vector.lower_ap(pv)],
                        outs=[nc.vector.lower_ap(acc_d[:, 6 * p_ : 6 * p_ + 6])],
                    )
                )
            for i in range(N_ACT):
                nc.scalar.activation(
                    out=scr,
                    in_=ps[:, P * i : P * (i + 1)],
                    func=mybir.ActivationFunctionType.Square,
                    accum_out=acc_a[:, i : i + 1],
                )
            nc.sync.dma_start(out=out_dv, in_=acc_d)
            nc.scalar.dma_start(out=out_a, in_=acc_a)

    fix_tail_sync(nc)
    split_excess_waits(nc)
    tune_const_memsets(nc)
    return nc


_CACHE = {}


def _prep(x: np.ndarray) -> np.ndarray:
    """Row-normalize to norm SCALE and quantize to fp8e4m3."""
    import ml_dtypes

    xf = np.ascontiguousarray(np.asarray(x, dtype=np.float32))
    n = np.sqrt(np.einsum("nd,nd->n", xf, xf))
    u = xf * (SCALE / (n + 1e-7))[:, None]
    return u.astype(ml_dtypes.float8_e4m3)


def _block_norms(aa: np.ndarray, ad: np.ndarray) -> list[float]:
    """Per-block ||A_b||_F^2 from one core's out_a [P, N_ACT] (Act accums)
    and out_dv [P, 6*N_PAIR] (parity-paired bn_stats), both float64."""
    norms = [0.0] * NBLK
    for i in range(N_ACT):
        norms[i] = float(aa[:, i].sum())
    for b in range(N_ACT, NBLK):
        p_, par = (b - N_ACT) // 2, (b - N_ACT) % 2
        st = ad[:, 6 * p_ + 3 * par : 6 * p_ + 3 * par + 3]
        norms[b] = float((st[:, 2] + st[:, 0] * st[:, 1] ** 2).sum())
    return norms


def kernel(feat_q: np.ndarray, feat_k: np.ndarray) -> np.ndarray:
    assert feat_q.shape == (N, D) and feat_k.shape == (N, D)

    if "nc" not in _CACHE:
        _CACHE["nc"] = build_program()
    nc = _CACHE["nc"]

    uq8 = _prep(feat_q)
    uk8 = _prep(feat_k)
    w8 = np.concatenate([uq8, uk8], axis=1)      # [N, 2D]; chunk i = 128 cols
    in_maps = []
    for c in range(N_CORES):
        cols = np.concatenate(
            [w8[:, P * ch : P * (ch + 1)] for ch in PHIS[c]], axis=1
        )
        in_maps.append({"x": np.ascontiguousarray(cols)})
    res = run_bass_kernel_spmd(nc, in_maps, list(range(N_CORES)))

    vals = {}
    for c in range(N_CORES):
        aa = np.asarray(res.results[c]["out_a"], dtype=np.float64)
        ad = np.asarray(res.results[c]["out_dv"], dtype=np.float64)
        norms = _block_norms(aa, ad)
        phi = PHIS[c]
        for pos, (u, v) in enumerate(PATTERN):
            x_, y_ = phi[u], phi[v]
            key = (min(x_, y_), max(x_, y_))
            if key not in vals:
                vals[key] = norms[pos]

    nq = D // P   # 8 chunks per tensor
    total = 0.0
    for i in range(2 * nq):
        for j in range(i, 2 * nq):
            v = vals[(i, j)]
            if (i < nq) == (j < nq):
                total += v if i == j else 2.0 * v
            else:
                total += -2.0 * v
    loss = total / (SCALE**4) / (N * (N - 1))
    return np.asarray(loss, dtype=np.float32)


if __name__ == "__main__":
    rng = np.random.default_rng(0)
    q = rng.standard_normal((N, D)).astype(np.float32)
    k = rng.standard_normal((N, D)).astype(np.float32)
    print("loss:", kernel(q, k))


# revision 3
# speedup vs baseline: 1.0119x; 1.0119x over previous
"""Trainium2 Bass kernel for nn_ConstLoss_22746146800082 (covering-design fp8).

loss * N(N-1) = ||Cq - Ck||_F^2 = ||Aqq||^2 + ||Akk||^2 - 2||Aqk||^2 with the
feature-space Grams of row-normalized features (see kernel.py v1).  v1 streamed
the FULL 8 MB (xq+xk) to every core (each core owning 128 Gram rows), which is
DMA-bound at ~23 us on the ~360 GB/s per-core DMA fabric.

v2 shards the Gram by 128x128 BLOCKS instead: the 2048 columns of W=[Qn|Kn]
form 16 chunks of 128; block (a,b) = chunk_a^T chunk_b needs only chunks a,b.
A covering design assigns 7 chunks (3.5 MB) to each core such that all
C(16,2)+16 chunk pairs appear on some core.  Every core runs the SAME program
(SPMD): it contracts a fixed 20-position slot-pair PATTERN (2 loops + 18
edges) over its 7 resident chunk slots; the host maps slots->chunks per core
(free gather), dedups duplicated pairs, and applies the +-2/+1 weights.

Per-core: DMA 3.5 MB (~10.2 us), PE 20 blocks x 1024 cyc = 20480 cyc
(~8.5 us @2.4GHz), tail = per-block sum-of-squares via DVE bn_stats (2 blocks
per instr via an interleaved stride view: even/odd parity stats = the two
blocks) and Act square+accum, one [128, 48] f32 result DMA.
"""

import numpy as np

import concourse.bass as bass
import concourse.mybir as mybir
import concourse.tile as tile
from concourse.vector_clock import ScopedClock
from concourse.bass_utils import run_bass_kernel_spmd

N_CORES = 8
N = 4096
D = 1024
P = 128
NSLOT = 7
COLS = NSLOT * P          # 896 resident feature columns per core
NC = N // P               # 32 sample chunks
NDC = NC // 2             # 16 DoubleRow double-chunks
SCALE = 32.0

F32 = mybir.dt.float32
FP8 = mybir.dt.float8e4
DR = mybir.MatmulPerfMode.DoubleRow

# ---- covering design (cover_search2.py): all 120 chunk pairs + 16 diags ---
EDGES = [(0, 1), (0, 2), (0, 3), (0, 4), (0, 5), (0, 6), (1, 3), (1, 4),
         (1, 5), (1, 6), (2, 3), (2, 4), (2, 5), (2, 6), (3, 6), (4, 6),
         (5, 6)]
LOOPS = [(0, 0), (1, 1)]
PATTERN = LOOPS + EDGES                      # 19 block positions
NBLK = len(PATTERN)
PHIS = [
    [5, 7, 14, 6, 1, 3, 10],
    [3, 14, 6, 9, 12, 13, 4],
    [15, 8, 0, 14, 6, 3, 11],
    [6, 1, 7, 14, 3, 8, 2],
    [4, 13, 11, 5, 2, 10, 15],
    [11, 12, 9, 7, 13, 1, 15],
    [0, 9, 4, 8, 1, 7, 13],
    [2, 10, 5, 0, 8, 9, 12],
]

N_ACT = 5                  # blocks 0..4 on Act (square + accum), close first
N_DVE = NBLK - N_ACT       # blocks 5..18 on DVE (7 parity-paired bn_stats)
N_PAIR = N_DVE // 2
OUTW_A = N_ACT
OUTW_D = 6 * N_PAIR

GROUP_SIZES = [2, 2, 4, 4, 4, 4, 4, 4, 2, 2]  # sample-chunks per DMA group


class _TC(tile.TileContext):
    """TileContext whose kernel-tail drain splits its semaphore waits across
    preceding sync-engine NOPs: this container's walrus build rejects a Drain
    carrying more than one sync wait ("Too many sync wait commands")."""

    def _drain_and_barrier(self, tick_clock, wait_clock):
        nc = self.nc
        probe = nc.sync.nop(nofuse=True)
        wait_clock.add_sem_waits(
            probe.ins, ScopedClock({None: tick_clock.global_clock})
        )
        waits = list(probe.ins.sync_info.on_wait or []) if probe.ins.sync_info else []
        if probe.ins.sync_info is not None:
            probe.ins.sync_info.on_wait = waits[:1]
        engines = [nc.vector, nc.scalar, nc.gpsimd, nc.tensor, nc.sync]
        for i, w in enumerate(waits[1:]):
            n2 = engines[i % len(engines)].nop(nofuse=True)
            n2.ins.sync_info = mybir.SyncInfo(on_wait=[w], on_update=[])
        nc.sync.drain()
        nc.all_engine_barrier()
        popped = nc._tile_sem_poison_stack.pop()
        assert popped is self._sem_poison
        nc.clear_and_free_semaphores(list(self.sems.allocated().values()))
        nc.all_engine_barrier()


MAX_WAITS_PER_INST = 1


def split_excess_waits(nc):
    """walrus rejects instructions carrying more than a couple of semaphore
    waits.  Hoist excess waits onto injected same-engine NOPs."""
    n = 0
    for f in nc.m.functions:
        for bb in f.blocks:
            insts = bb.instructions
            out = []
            changed = False
            for ins in insts:
                si = ins.sync_info
                waits = list(si.on_wait or []) if si is not None else []
                while len(waits) > MAX_WAITS_PER_INST:
                    take = waits[:MAX_WAITS_PER_INST]
                    waits = waits[MAX_WAITS_PER_INST:]
                    nop = mybir.InstNoOp(name=f"I-waitsplit-{n}", ins=[], outs=[])
                    n += 1
                    nop.engine = ins.engine
                    nop.sync_info = mybir.SyncInfo(on_wait=take, on_update=[])
                    out.append(nop)
                    changed = True
                if changed and si is not None:
                    si.on_wait = waits
                out.append(ins)
            if changed:
                bb.instructions = out
    return n


def tune_const_memsets(nc):
    """Drop unused const-tile memsets from the pre-barrier preamble; push the
    used ones past the entry barrier (their readers are in the kernel tail)."""
    used = set()
    for f in nc.m.functions:
        for bb in f.blocks:
            for ins in bb.instructions:
                for a in ins.ins:
                    m = getattr(a, "memref", None)
                    if m:
                        used.add(m)
    for f in nc.m.functions:
        if len(f.blocks) < 2:
            continue
        bb0, bb1 = f.blocks[0], f.blocks[1]
        keep, moved = [], []
        for ins in bb0.instructions:
            if ins.opcode == "Memset" and ins.outs:
                m = getattr(ins.outs[0], "memref", "")
                if m.startswith("const-"):
                    if m in used:
                        moved.append(ins)
                    continue
            keep.append(ins)
        if moved or len(keep) != len(bb0.instructions):
            bb0.instructions = keep
            bb1.instructions = moved + bb1.instructions


def fix_tail_sync(nc):
    """Tile emits conservative tail sync: every psum reader waits for ALL
    matmuls (tile-granular RAW), the first Act square waits on all DVE
    bn_stats (clock-compression proxy), and same-engine sem self-chains add
    ~160ns per instruction.  Rewrite the tail waits to precise PE ticks:
    psum region b's last write is matmul tick 300 + b + 1 (15 full dc rounds
    of NBLK matmuls, then the final-dc matmuls in block order)."""
    insts = [i for f in nc.m.functions for bb in f.blocks for i in bb.instructions]
    matmuls = [i for i in insts if i.opcode == "Matmult"]
    n_mm = len(matmuls)
    assert n_mm == NDC * NBLK + 1, n_mm  # +1 p-state warm-up matmul
    # find the PE completion-sem (the update attached to matmuls)
    pe_upd = None
    for i in matmuls:
        si = i.sync_info
        if si and si.on_update:
            for u in si.on_update:
                if u.ant_name and u.ant_name.startswith("PE"):
                    pe_upd = u
    assert pe_upd is not None

    def pe_wait(tick):
        return mybir.SyncWait(
            sync_type="semaphore",
            id=pe_upd.id,
            ant_name=pe_upd.ant_name,
            wait_mode="sem-ge-imm",
            wait_value=tick,
        )

    base = n_mm - NBLK  # matmul ticks before the final dc round
    bn = [i for i in insts if i.opcode == "BNStats"]
    assert len(bn) == N_PAIR
    for j, i in enumerate(bn):
        bmax = N_ACT + 2 * j + 1
        i.sync_info.on_wait = [pe_wait(base + bmax + 1)]
    acts = [i for i in insts if i.opcode == "Activation"]
    assert len(acts) == N_ACT
    # first Act square: precise PE tick covering all Act blocks (strictly
    # correct for the whole chain); later ones keep the Act self-chain, which
    # also serializes the shared hw accumulator register.
    acts[0].sync_info.on_wait = [pe_wait(base + N_ACT)]


def fix_prepared_out(nc):
    """Tile routes each Pool-engine DMA prep to a DMASW semaphore lane and
    the kernel-exit drain waits for those lane sems to reach 16, but the
    completion sem baked into a prepare_only descriptor is the user-passed
    `sem=` handle.  Point each prep's completion update at its assigned
    DMASW lane sem (found from the drain's own waits, in round-robin order)
    so the hardware DMA-complete increments what the drain is waiting on."""
    insts = [i for f in nc.m.functions for bb in f.blocks for i in bb.instructions]
    lane_sems = {}
    for i in insts:
        si = i.sync_info
        for w in (si.on_wait or []) if si else []:
            if w.ant_name and w.ant_name.startswith("DMASW"):
                lane = int(w.ant_name[5 : w.ant_name.index("_")])
                lane_sems[lane] = (w.id, w.ant_name)
    preps = [i for i in insts if i.opcode == "KVWritebackAnt"]
    assert len(preps) == len(lane_sems), (len(preps), lane_sems)
    for j, i in enumerate(preps):
        sid, sname = lane_sems[j]
        for u in i.sync_info.on_update or []:
            if u.ant_name == "out_dma_sem":
                u.id = sid
                u.ant_name = sname


def build_program(sim_mode: bool = False):
    nc = bass.Bass(
        "TRN2", target_bir_lowering=False, debug=False, num_devices=N_CORES
    )
    x = nc.dram_tensor("x", [N, COLS], FP8, kind="ExternalInput").ap()
    out_a = nc.dram_tensor("out_a", [P, OUTW_A], F32, kind="ExternalOutput").ap()
    out_dv = nc.dram_tensor("out_dv", [P, OUTW_D], F32, kind="ExternalOutput").ap()

    with _TC(nc) as tc:
        with (
            tc.tile_pool(name="stream", bufs=1) as stream,
            tc.tile_pool(name="tail", bufs=1) as tail,
            tc.tile_pool(name="psum", bufs=1, space="PSUM") as psum,
        ):
            ps = psum.tile([P, NBLK * P], F32, name="ps", tag="ps")

            acc_a = tail.tile([P, OUTW_A], F32, name="acc_a")
            acc_d = tail.tile([P, OUTW_D], F32, name="acc_d")

            # PE p-state warm-up: one tiny all-zeros matmul right after the
            # preamble starts the 3us ramp clock, so the real matmuls (first
            # data lands ~3.6us) run at full 2.4 GHz from the start.  It
            # doubles as bank 0's start=True zeroing (contributes exact 0s),
            # so blocks 0..3 accumulate with start=False below.
            warm = tail.tile([P, 2, P], FP8, name="warm")
            nc.gpsimd.memset(warm, 0)
            nc.tensor.matmul(
                ps[:, 0:P], lhsT=warm, rhs=warm,
                start=True, stop=False, perf_mode=DR,
            )

            xr = x.rearrange("(c p) d -> p c d", p=P)
            off = 0
            dc = 0
            for g, m in enumerate(GROUP_SIZES):
                tg = stream.tile([P, m, COLS], FP8, name=f"s{g}", tag=f"s{g}")
                nc.sync.dma_start(out=tg, in_=xr[:, off : off + m, :])
                for jj in range(m // 2):
                    pr = slice(2 * jj, 2 * jj + 2)
                    for pos, (u, v) in enumerate(PATTERN):
                        # start=True zeroes the ENTIRE psum bank, so only the
                        # first block of each 4-block bank may carry it; the
                        # other blocks accumulate onto the freshly-zeroed bank.
                        nc.tensor.matmul(
                            ps[:, P * pos : P * (pos + 1)],
                            lhsT=tg[:, pr, P * u : P * (u + 1)],
                            rhs=tg[:, pr, P * v : P * (v + 1)],
                            start=(dc == 0 and pos % 4 == 0 and pos > 0),
                            stop=(dc == NDC - 1),
                            perf_mode=DR,
                        )
                    dc += 1
                off += m

            # ---- tail: per-block sum(A^2) partials --------------------------
            # Act: blocks 0..N_ACT-1 (their psum regions close first), square
            # activation with accumulator -> acc_a[:, b].
            # DVE: remaining blocks via parity-paired bn_stats: the input AP
            # interleaves two 128-wide blocks at element granularity (outer
            # dim stride 1, inner dim stride 128), so the even-parity stats
            # are the first block and the odd-parity stats the second; host
            # recovers sum(x^2) = M2 + n*mean^2 per parity.  Separate output
            # tiles per engine so the two reduce streams proceed
            # independently.
            scr = tail.tile([P, P], F32, name="scr")
            for p_ in range(N_PAIR):
                b0 = N_ACT + 2 * p_
                pv = ps[:, P * b0 : P * (b0 + 2)].rearrange(
                    "p (b i) -> p i b", b=2
                )
                # emit InstBNStats directly: the bass wrapper insists batched
                # outputs for multi-dim inputs, but walrus only accepts the
                # plain 6-per-partition form; the DVE streams the input AP in
                # order, so this interleaved view alternates the two blocks
                # and the even/odd parity stats separate them again.
                nc.vector.add_instruction(
                    mybir.InstBNStats(
                        name=nc.get_next_instruction_name(),
                        ins=[nc.vector.lower_ap(pv)],
                        outs=[nc.vector.lower_ap(acc_d[:, 6 * p_ : 6 * p_ + 6])],
                    )
                )
            for i in range(N_ACT):
                nc.scalar.activation(
                    out=scr,
                    in_=ps[:, P * i : P * (i + 1)],
                    func=mybir.ActivationFunctionType.Square,
                    accum_out=acc_a[:, i : i + 1],
                )
            nc.sync.dma_start(out=out_dv, in_=acc_d)
            nc.scalar.dma_start(out=out_a, in_=acc_a)

    fix_tail_sync(nc)
    split_excess_waits(nc)
    tune_const_memsets(nc)
    return nc


_CACHE = {}


def _prep(x: np.ndarray) -> np.ndarray:
    """Row-normalize to norm SCALE and quantize to fp8e4m3."""
    import ml_dtypes

    xf = np.ascontiguousarray(np.asarray(x, dtype=np.float32))
    n = np.sqrt(np.einsum("nd,nd->n", xf, xf))
    u = xf * (SCALE / (n + 1e-7))[:, None]
    return u.astype(ml_dtypes.float8_e4m3)


def _block_norms(aa: np.ndarray, ad: np.ndarray) -> list[float]:
    """Per-block ||A_b||_F^2 from one core's out_a [P, N_ACT] (Act accums)
    and out_dv [P, 6*N_PAIR] (parity-paired bn_stats), both float64."""
    norms = [0.0] * NBLK
    for i in range(N_ACT):
        norms[i] = float(aa[:, i].sum())
    for b in range(N_ACT, NBLK):
        p_, par = (b - N_ACT) // 2, (b - N_ACT) % 2
        st = ad[:, 6 * p_ + 3 * par : 6 * p_ + 3 * par + 3]
        norms[b] = float((st[:, 2] + st[:, 0] * st[:, 1] ** 2).sum())
    return norms


def kernel(feat_q: np.ndarray, feat_k: np.ndarray) -> np.ndarray:
    assert feat_q.shape == (N, D) and feat_k.shape == (N, D)

    if "nc" not in _CACHE:
        _CACHE["nc"] = build_program()
    nc = _CACHE["nc"]

    uq8 = _prep(feat_q)
    uk8 = _prep(feat_k)
    w8 = np.concatenate([uq8, uk8], axis=1)      # [N, 2D]; chunk i = 128 cols
    in_maps = []
    for c in range(N_CORES):
        cols = np.concatenate(
            [w8[:, P * ch : P * (ch + 1)] for ch in PHIS[c]], axis=1
        )
        in_maps.append({"x": np.ascontiguousarray(cols)})
    res = run_bass_kernel_spmd(nc, in_maps, list(range(N_CORES)))

    vals = {}
    for c in range(N_CORES):
        aa = np.asarray(res.results[c]["out_a"], dtype=np.float64)
        ad = np.asarray(res.results[c]["out_dv"], dtype=np.float64)
        norms = _block_norms(aa, ad)
        phi = PHIS[c]
        for pos, (u, v) in enumerate(PATTERN):
            x_, y_ = phi[u], phi[v]
            key = (min(x_, y_), max(x_, y_))
            if key not in vals:
                vals[key] = norms[pos]

    nq = D // P   # 8 chunks per tensor
    total = 0.0
    for i in range(2 * nq):
        for j in range(i, 2 * nq):
            v = vals[(i, j)]
            if (i < nq) == (j < nq):
                total += v if i == j else 2.0 * v
            else:
                total += -2.0 * v
    loss = total / (SCALE**4) / (N * (N - 1))
    return np.asarray(loss, dtype=np.float32)


if __name__ == "__main__":
    rng = np.random.default_rng(0)
    q = rng.standard_normal((N, D)).astype(np.float32)
    k = rng.standard_normal((N, D)).astype(np.float32)
    print("loss:", kernel(q, k))


# revision 5
# speedup vs baseline: 1.0300x; 1.0179x over previous
"""Trainium2 Bass kernel for nn_ConstLoss_22746146800082 (covering-design fp8).

loss * N(N-1) = ||Cq - Ck||_F^2 = ||Aqq||^2 + ||Akk||^2 - 2||Aqk||^2 with the
feature-space Grams of row-normalized features (see kernel.py v1).  v1 streamed
the FULL 8 MB (xq+xk) to every core (each core owning 128 Gram rows), which is
DMA-bound at ~23 us on the ~360 GB/s per-core DMA fabric.

v2 shards the Gram by 128x128 BLOCKS instead: the 2048 columns of W=[Qn|Kn]
form 16 chunks of 128; block (a,b) = chunk_a^T chunk_b needs only chunks a,b.
A covering design assigns 7 chunks (3.5 MB) to each core such that all
C(16,2)+16 chunk pairs appear on some core.  Every core runs the SAME program
(SPMD): it contracts a fixed 19-position slot-pair PATTERN (2 loops + 17
edges) over its 7 resident chunk slots; the host maps slots->chunks per core
(free gather), dedups duplicated pairs, and applies the +-2/+1 weights.

Per-core: DMA 3.5 MB (~10.2 us, the binding resource), PE 19 blocks x 1024
cyc (~8.1 us @2.4GHz, warmed to full p-state by an early dummy matmul that
also bank-zeroes psum bank 0 -- matmul start=True zeroes the WHOLE psum
bank, so only the first block per bank carries it), tail = per-block
sum-of-squares via parity-paired DVE bn_stats (two 128-wide blocks
interleaved at element stride so the even/odd parity stats separate them)
plus Act square+accum singles, one [128, 47] f32 result DMA.  Tail waits are
rewritten post-build to precise per-psum-region PE ticks (fix_tail_sync) so
both reduce engines start the moment their regions close.
"""

import numpy as np

import concourse.bass as bass
import concourse.mybir as mybir
import concourse.tile as tile
from concourse.vector_clock import ScopedClock
from concourse.bass_utils import run_bass_kernel_spmd

N_CORES = 8
N = 4096
D = 1024
P = 128
NSLOT = 7
COLS = NSLOT * P          # 896 resident feature columns per core
NC = N // P               # 32 sample chunks
NDC = NC // 2             # 16 DoubleRow double-chunks
SCALE = 32.0

F32 = mybir.dt.float32
FP8 = mybir.dt.float8e4
DR = mybir.MatmulPerfMode.DoubleRow

# ---- covering design (cover_search2.py): all 120 chunk pairs + 16 diags ---
EDGES = [(0, 1), (0, 2), (0, 3), (0, 4), (0, 5), (0, 6), (1, 3), (1, 4),
         (1, 5), (1, 6), (2, 3), (2, 4), (2, 5), (2, 6), (3, 6), (4, 6),
         (5, 6)]
LOOPS = [(0, 0), (1, 1)]
PATTERN = LOOPS + EDGES                      # 19 block positions
NBLK = len(PATTERN)
PHIS = [
    [5, 7, 14, 6, 1, 3, 10],
    [3, 14, 6, 9, 12, 13, 4],
    [15, 8, 0, 14, 6, 3, 11],
    [6, 1, 7, 14, 3, 8, 2],
    [4, 13, 11, 5, 2, 10, 15],
    [11, 12, 9, 7, 13, 1, 15],
    [0, 9, 4, 8, 1, 7, 13],
    [2, 10, 5, 0, 8, 9, 12],
]

N_ACT = 5                  # blocks 0..4 on Act (square + accum), close first
N_DVE = NBLK - N_ACT       # blocks 5..18 on DVE (7 parity-paired bn_stats)
N_PAIR = N_DVE // 2
OUTW_A = N_ACT
OUTW_D = 6 * N_PAIR

GROUP_SIZES = [2, 2, 4, 4, 4, 4, 4, 4, 2, 2]  # sample-chunks per DMA group


class _TC(tile.TileContext):
    """TileContext whose kernel-tail drain splits its semaphore waits across
    preceding sync-engine NOPs: this container's walrus build rejects a Drain
    carrying more than one sync wait ("Too many sync wait commands")."""

    def _drain_and_barrier(self, tick_clock, wait_clock):
        nc = self.nc
        probe = nc.sync.nop(nofuse=True)
        wait_clock.add_sem_waits(
            probe.ins, ScopedClock({None: tick_clock.global_clock})
        )
        waits = list(probe.ins.sync_info.on_wait or []) if probe.ins.sync_info else []
        if probe.ins.sync_info is not None:
            probe.ins.sync_info.on_wait = waits[:1]
        engines = [nc.vector, nc.scalar, nc.gpsimd, nc.tensor, nc.sync]
        for i, w in enumerate(waits[1:]):
            n2 = engines[i % len(engines)].nop(nofuse=True)
            n2.ins.sync_info = mybir.SyncInfo(on_wait=[w], on_update=[])
        nc.sync.drain()
        nc.all_engine_barrier()
        popped = nc._tile_sem_poison_stack.pop()
        assert popped is self._sem_poison
        nc.clear_and_free_semaphores(list(self.sems.allocated().values()))
        nc.all_engine_barrier()


MAX_WAITS_PER_INST = 1


def split_excess_waits(nc):
    """walrus rejects instructions carrying more than a couple of semaphore
    waits.  Hoist excess waits onto injected same-engine NOPs."""
    n = 0
    for f in nc.m.functions:
        for bb in f.blocks:
            insts = bb.instructions
            out = []
            changed = False
            for ins in insts:
                si = ins.sync_info
                waits = list(si.on_wait or []) if si is not None else []
                while len(waits) > MAX_WAITS_PER_INST:
                    take = waits[:MAX_WAITS_PER_INST]
                    waits = waits[MAX_WAITS_PER_INST:]
                    nop = mybir.InstNoOp(name=f"I-waitsplit-{n}", ins=[], outs=[])
                    n += 1
                    nop.engine = ins.engine
                    nop.sync_info = mybir.SyncInfo(on_wait=take, on_update=[])
                    out.append(nop)
                    changed = True
                if changed and si is not None:
                    si.on_wait = waits
                out.append(ins)
            if changed:
                bb.instructions = out
    return n


def tune_const_memsets(nc):
    """Drop unused const-tile memsets from the pre-barrier preamble; push the
    used ones past the entry barrier (their readers are in the kernel tail)."""
    used = set()
    for f in nc.m.functions:
        for bb in f.blocks:
            for ins in bb.instructions:
                for a in ins.ins:
                    m = getattr(a, "memref", None)
                    if m:
                        used.add(m)
    for f in nc.m.functions:
        if len(f.blocks) < 2:
            continue
        bb0, bb1 = f.blocks[0], f.blocks[1]
        keep, moved = [], []
        for ins in bb0.instructions:
            if ins.opcode == "Memset" and ins.outs:
                m = getattr(ins.outs[0], "memref", "")
                if m.startswith("const-"):
                    if m in used:
                        moved.append(ins)
                    continue
            keep.append(ins)
        if moved or len(keep) != len(bb0.instructions):
            bb0.instructions = keep
            bb1.instructions = moved + bb1.instructions


def fix_tail_sync(nc):
    """Tile emits conservative tail sync: every psum reader waits for ALL
    matmuls (tile-granular RAW), the first Act square waits on all DVE
    bn_stats (clock-compression proxy), and same-engine sem self-chains add
    ~160ns per instruction.  Rewrite the tail waits to precise PE ticks:
    psum region b's last write is matmul tick 300 + b + 1 (15 full dc rounds
    of NBLK matmuls, then the final-dc matmuls in block order)."""
    insts = [i for f in nc.m.functions for bb in f.blocks for i in bb.instructions]
    matmuls = [i for i in insts if i.opcode == "Matmult"]
    n_mm = len(matmuls)
    assert n_mm == NDC * NBLK + 1, n_mm  # +1 p-state warm-up matmul
    # find the PE completion-sem (the update attached to matmuls)
    pe_upd = None
    for i in matmuls:
        si = i.sync_info
        if si and si.on_update:
            for u in si.on_update:
                if u.ant_name and u.ant_name.startswith("PE"):
                    pe_upd = u
    assert pe_upd is not None

    def pe_wait(tick):
        return mybir.SyncWait(
            sync_type="semaphore",
            id=pe_upd.id,
            ant_name=pe_upd.ant_name,
            wait_mode="sem-ge-imm",
            wait_value=tick,
        )

    base = n_mm - NBLK  # matmul ticks before the final dc round
    bn = [i for i in insts if i.opcode == "BNStats"]
    assert len(bn) == N_PAIR
    for j, i in enumerate(bn):
        bmax = N_ACT + 2 * j + 1
        i.sync_info.on_wait = [pe_wait(base + bmax + 1)]
    acts = [i for i in insts if i.opcode == "Activation"]
    assert len(acts) == N_ACT
    # first Act square: precise PE tick covering all Act blocks (strictly
    # correct for the whole chain); later ones keep the Act self-chain, which
    # also serializes the shared hw accumulator register.
    acts[0].sync_info.on_wait = [pe_wait(base + N_ACT)]


def fix_prepared_out(nc):
    """Tile routes each Pool-engine DMA prep to a DMASW semaphore lane and
    the kernel-exit drain waits for those lane sems to reach 16, but the
    completion sem baked into a prepare_only descriptor is the user-passed
    `sem=` handle.  Point each prep's completion update at its assigned
    DMASW lane sem (found from the drain's own waits, in round-robin order)
    so the hardware DMA-complete increments what the drain is waiting on."""
    insts = [i for f in nc.m.functions for bb in f.blocks for i in bb.instructions]
    lane_sems = {}
    for i in insts:
        si = i.sync_info
        for w in (si.on_wait or []) if si else []:
            if w.ant_name and w.ant_name.startswith("DMASW"):
                lane = int(w.ant_name[5 : w.ant_name.index("_")])
                lane_sems[lane] = (w.id, w.ant_name)
    preps = [i for i in insts if i.opcode == "KVWritebackAnt"]
    assert len(preps) == len(lane_sems), (len(preps), lane_sems)
    for j, i in enumerate(preps):
        sid, sname = lane_sems[j]
        for u in i.sync_info.on_update or []:
            if u.ant_name == "out_dma_sem":
                u.id = sid
                u.ant_name = sname


def build_program(sim_mode: bool = False):
    nc = bass.Bass(
        "TRN2", target_bir_lowering=False, debug=False, num_devices=N_CORES
    )
    x = nc.dram_tensor("x", [N, COLS], FP8, kind="ExternalInput").ap()
    out_c = nc.dram_tensor("out_c", [P, OUTW_A + OUTW_D], F32, kind="ExternalOutput").ap()

    with _TC(nc) as tc:
        with (
            tc.tile_pool(name="stream", bufs=1) as stream,
            tc.tile_pool(name="tail", bufs=1) as tail,
            tc.tile_pool(name="psum", bufs=1, space="PSUM") as psum,
        ):
            ps = psum.tile([P, NBLK * P], F32, name="ps", tag="ps")

            acc = tail.tile([P, OUTW_A + OUTW_D], F32, name="acc")
            acc_a = acc[:, 0:OUTW_A]
            acc_d = acc[:, OUTW_A : OUTW_A + OUTW_D]

            # PE p-state warm-up: one tiny all-zeros matmul right after the
            # preamble starts the 3us ramp clock, so the real matmuls (first
            # data lands ~3.6us) run at full 2.4 GHz from the start.  It
            # doubles as bank 0's start=True zeroing (contributes exact 0s),
            # so blocks 0..3 accumulate with start=False below.
            warm = tail.tile([P, 2, P], FP8, name="warm")
            nc.gpsimd.memset(warm, 0)
            nc.tensor.matmul(
                ps[:, 0:P], lhsT=warm, rhs=warm,
                start=True, stop=False, perf_mode=DR,
            )

            xr = x.rearrange("(c p) d -> p c d", p=P)
            off = 0
            dc = 0
            for g, m in enumerate(GROUP_SIZES):
                tg = stream.tile([P, m, COLS], FP8, name=f"s{g}", tag=f"s{g}")
                nc.sync.dma_start(out=tg, in_=xr[:, off : off + m, :])
                for jj in range(m // 2):
                    pr = slice(2 * jj, 2 * jj + 2)
                    for pos, (u, v) in enumerate(PATTERN):
                        # start=True zeroes the ENTIRE psum bank, so only the
                        # first block of each 4-block bank may carry it; the
                        # other blocks accumulate onto the freshly-zeroed bank.
                        nc.tensor.matmul(
                            ps[:, P * pos : P * (pos + 1)],
                            lhsT=tg[:, pr, P * u : P * (u + 1)],
                            rhs=tg[:, pr, P * v : P * (v + 1)],
                            start=(dc == 0 and pos % 4 == 0 and pos > 0),
                            stop=(dc == NDC - 1),
                            perf_mode=DR,
                        )
                    dc += 1
                off += m

            # ---- tail: per-block sum(A^2) partials --------------------------
            # Act: blocks 0..N_ACT-1 (their psum regions close first), square
            # activation with accumulator -> acc_a[:, b].
            # DVE: remaining blocks via parity-paired bn_stats: the input AP
            # interleaves two 128-wide blocks at element granularity (outer
            # dim stride 1, inner dim stride 128), so the even-parity stats
            # are the first block and the odd-parity stats the second; host
            # recovers sum(x^2) = M2 + n*mean^2 per parity.  Separate output
            # tiles per engine so the two reduce streams proceed
            # independently.
            scr = tail.tile([P, P], F32, name="scr")
            for p_ in range(N_PAIR):
                b0 = N_ACT + 2 * p_
                pv = ps[:, P * b0 : P * (b0 + 2)].rearrange(
                    "p (b i) -> p i b", b=2
                )
                # emit InstBNStats directly: the bass wrapper insists batched
                # outputs for multi-dim inputs, but walrus only accepts the
                # plain 6-per-partition form; the DVE streams the input AP in
                # order, so this interleaved view alternates the two blocks
                # and the even/odd parity stats separate them again.
                nc.vector.add_instruction(
                    mybir.InstBNStats(
                        name=nc.get_next_instruction_name(),
                        ins=[nc.vector.lower_ap(pv)],
                        outs=[nc.vector.lower_ap(acc_d[:, 6 * p_ : 6 * p_ + 6])],
                    )
                )
            for i in range(N_ACT):
                nc.scalar.activation(
                    out=scr,
                    in_=ps[:, P * i : P * (i + 1)],
                    func=mybir.ActivationFunctionType.Square,
                    accum_out=acc_a[:, i : i + 1],
                )
            nc.sync.dma_start(out=out_c, in_=acc)

    fix_tail_sync(nc)
    split_excess_waits(nc)
    tune_const_memsets(nc)
    return nc


_CACHE = {}


def _prep(x: np.ndarray) -> np.ndarray:
    """Row-normalize to norm SCALE and quantize to fp8e4m3."""
    import ml_dtypes

    xf = np.ascontiguousarray(np.asarray(x, dtype=np.float32))
    n = np.sqrt(np.einsum("nd,nd->n", xf, xf))
    u = xf * (SCALE / (n + 1e-7))[:, None]
    return u.astype(ml_dtypes.float8_e4m3)


def _block_norms(aa: np.ndarray, ad: np.ndarray) -> list[float]:
    """Per-block ||A_b||_F^2 from one core's out_a [P, N_ACT] (Act accums)
    and out_dv [P, 6*N_PAIR] (parity-paired bn_stats), both float64."""
    norms = [0.0] * NBLK
    for i in range(N_ACT):
        norms[i] = float(aa[:, i].sum())
    for b in range(N_ACT, NBLK):
        p_, par = (b - N_ACT) // 2, (b - N_ACT) % 2
        st = ad[:, 6 * p_ + 3 * par : 6 * p_ + 3 * par + 3]
        norms[b] = float((st[:, 2] + st[:, 0] * st[:, 1] ** 2).sum())
    return norms


def kernel(feat_q: np.ndarray, feat_k: np.ndarray) -> np.ndarray:
    assert feat_q.shape == (N, D) and feat_k.shape == (N, D)

    if "nc" not in _CACHE:
        _CACHE["nc"] = build_program()
    nc = _CACHE["nc"]

    uq8 = _prep(feat_q)
    uk8 = _prep(feat_k)
    w8 = np.concatenate([uq8, uk8], axis=1)      # [N, 2D]; chunk i = 128 cols
    in_maps = []
    for c in range(N_CORES):
        cols = np.concatenate(
            [w8[:, P * ch : P * (ch + 1)] for ch in PHIS[c]], axis=1
        )
        in_maps.append({"x": np.ascontiguousarray(cols)})
    res = run_bass_kernel_spmd(nc, in_maps, list(range(N_CORES)))

    vals = {}
    for c in range(N_CORES):
        ac = np.asarray(res.results[c]["out_c"], dtype=np.float64)
        aa, ad = ac[:, :OUTW_A], ac[:, OUTW_A:]
        norms = _block_norms(aa, ad)
        phi = PHIS[c]
        for pos, (u, v) in enumerate(PATTERN):
            x_, y_ = phi[u], phi[v]
            key = (min(x_, y_), max(x_, y_))
            if key not in vals:
                vals[key] = norms[pos]

    nq = D // P   # 8 chunks per tensor
    total = 0.0
    for i in range(2 * nq):
        for j in range(i, 2 * nq):
            v = vals[(i, j)]
            if (i < nq) == (j < nq):
                total += v if i == j else 2.0 * v
            else:
                total += -2.0 * v
    loss = total / (SCALE**4) / (N * (N - 1))
    return np.asarray(loss, dtype=np.float32)


if __name__ == "__main__":
    rng = np.random.default_rng(0)
    q = rng.standard_normal((N, D)).astype(np.float32)
    k = rng.standard_normal((N, D)).astype(np.float32)
    print("loss:", kernel(q, k))


# revision 6
# speedup vs baseline: 1.0440x; 1.0136x over previous
"""Trainium2 Bass kernel for nn_ConstLoss_22746146800082 (covering-design fp8).

loss * N(N-1) = ||Cq - Ck||_F^2 = ||Aqq||^2 + ||Akk||^2 - 2||Aqk||^2 with the
feature-space Grams of row-normalized features (see kernel.py v1).  v1 streamed
the FULL 8 MB (xq+xk) to every core (each core owning 128 Gram rows), which is
DMA-bound at ~23 us on the ~360 GB/s per-core DMA fabric.

v2 shards the Gram by 128x128 BLOCKS instead: the 2048 columns of W=[Qn|Kn]
form 16 chunks of 128; block (a,b) = chunk_a^T chunk_b needs only chunks a,b.
A covering design assigns 7 chunks (3.5 MB) to each core such that all
C(16,2)+16 chunk pairs appear on some core.  Every core runs the SAME program
(SPMD): it contracts a fixed 19-position slot-pair PATTERN (2 loops + 17
edges) over its 7 resident chunk slots; the host maps slots->chunks per core
(free gather), dedups duplicated pairs, and applies the +-2/+1 weights.

Per-core: DMA 3.5 MB (~10.2 us, the binding resource), PE 19 blocks x 1024
cyc (~8.1 us @2.4GHz, warmed to full p-state by an early dummy matmul that
also bank-zeroes psum bank 0 -- matmul start=True zeroes the WHOLE psum
bank, so only the first block per bank carries it), tail = per-block
sum-of-squares via parity-paired DVE bn_stats (two 128-wide blocks
interleaved at element stride so the even/odd parity stats separate them)
plus Act square+accum singles, one [128, 47] f32 result DMA.  Tail waits are
rewritten post-build to precise per-psum-region PE ticks (fix_tail_sync) so
both reduce engines start the moment their regions close.
"""

import numpy as np

import concourse.bass as bass
import concourse.mybir as mybir
import concourse.tile as tile
from concourse.vector_clock import ScopedClock
from concourse.bass_utils import run_bass_kernel_spmd

N_CORES = 8
N = 4096
D = 1024
P = 128
NSLOT = 7
COLS = NSLOT * P          # 896 resident feature columns per core
NC = N // P               # 32 sample chunks
NDC = NC // 2             # 16 DoubleRow double-chunks
SCALE = 32.0

F32 = mybir.dt.float32
FP8 = mybir.dt.float8e4
DR = mybir.MatmulPerfMode.DoubleRow

# ---- covering design (cover_search2.py): all 120 chunk pairs + 16 diags ---
EDGES = [(0, 1), (0, 2), (0, 3), (0, 4), (0, 5), (0, 6), (1, 3), (1, 4),
         (1, 5), (1, 6), (2, 3), (2, 4), (2, 5), (2, 6), (3, 6), (4, 6),
         (5, 6)]
LOOPS = [(0, 0), (1, 1)]
PATTERN = LOOPS + EDGES                      # 19 block positions
NBLK = len(PATTERN)
PHIS = [
    [5, 7, 14, 6, 1, 3, 10],
    [3, 14, 6, 9, 12, 13, 4],
    [15, 8, 0, 14, 6, 3, 11],
    [6, 1, 7, 14, 3, 8, 2],
    [4, 13, 11, 5, 2, 10, 15],
    [11, 12, 9, 7, 13, 1, 15],
    [0, 9, 4, 8, 1, 7, 13],
    [2, 10, 5, 0, 8, 9, 12],
]

N_ACT = 5                  # blocks 0..4 on Act (square + accum), close first
N_DVE = NBLK - N_ACT       # blocks 5..18 on DVE (7 parity-paired bn_stats)
N_PAIR = N_DVE // 2
OUTW_A = N_ACT
OUTW_D = 6 * N_PAIR

GROUP_SIZES = [2, 2, 4, 4, 4, 4, 2, 2, 2, 2, 2, 2]  # sample-chunks per group:
# small head groups start PE early; small tail groups keep the PE backlog
# short so the final double-chunk round (and the tail reduce) starts sooner.


class _TC(tile.TileContext):
    """TileContext whose kernel-tail drain splits its semaphore waits across
    preceding sync-engine NOPs: this container's walrus build rejects a Drain
    carrying more than one sync wait ("Too many sync wait commands")."""

    def _drain_and_barrier(self, tick_clock, wait_clock):
        nc = self.nc
        probe = nc.sync.nop(nofuse=True)
        wait_clock.add_sem_waits(
            probe.ins, ScopedClock({None: tick_clock.global_clock})
        )
        waits = list(probe.ins.sync_info.on_wait or []) if probe.ins.sync_info else []
        if probe.ins.sync_info is not None:
            probe.ins.sync_info.on_wait = waits[:1]
        engines = [nc.vector, nc.scalar, nc.gpsimd, nc.tensor, nc.sync]
        for i, w in enumerate(waits[1:]):
            n2 = engines[i % len(engines)].nop(nofuse=True)
            n2.ins.sync_info = mybir.SyncInfo(on_wait=[w], on_update=[])
        nc.sync.drain()
        nc.all_engine_barrier()
        popped = nc._tile_sem_poison_stack.pop()
        assert popped is self._sem_poison
        nc.clear_and_free_semaphores(list(self.sems.allocated().values()))
        nc.all_engine_barrier()


MAX_WAITS_PER_INST = 1


def split_excess_waits(nc):
    """walrus rejects instructions carrying more than a couple of semaphore
    waits.  Hoist excess waits onto injected same-engine NOPs."""
    n = 0
    for f in nc.m.functions:
        for bb in f.blocks:
            insts = bb.instructions
            out = []
            changed = False
            for ins in insts:
                si = ins.sync_info
                waits = list(si.on_wait or []) if si is not None else []
                while len(waits) > MAX_WAITS_PER_INST:
                    take = waits[:MAX_WAITS_PER_INST]
                    waits = waits[MAX_WAITS_PER_INST:]
                    nop = mybir.InstNoOp(name=f"I-waitsplit-{n}", ins=[], outs=[])
                    n += 1
                    nop.engine = ins.engine
                    nop.sync_info = mybir.SyncInfo(on_wait=take, on_update=[])
                    out.append(nop)
                    changed = True
                if changed and si is not None:
                    si.on_wait = waits
                out.append(ins)
            if changed:
                bb.instructions = out
    return n


def tune_const_memsets(nc):
    """Drop unused const-tile memsets from the pre-barrier preamble; push the
    used ones past the entry barrier (their readers are in the kernel tail)."""
    used = set()
    for f in nc.m.functions:
        for bb in f.blocks:
            for ins in bb.instructions:
                for a in ins.ins:
                    m = getattr(a, "memref", None)
                    if m:
                        used.add(m)
    for f in nc.m.functions:
        if len(f.blocks) < 2:
            continue
        bb0, bb1 = f.blocks[0], f.blocks[1]
        keep, moved = [], []
        for ins in bb0.instructions:
            if ins.opcode == "Memset" and ins.outs:
                m = getattr(ins.outs[0], "memref", "")
                if m.startswith("const-"):
                    if m in used:
                        moved.append(ins)
                    continue
            keep.append(ins)
        if moved or len(keep) != len(bb0.instructions):
            bb0.instructions = keep
            bb1.instructions = moved + bb1.instructions


def fix_tail_sync(nc):
    """Tile emits conservative tail sync: every psum reader waits for ALL
    matmuls (tile-granular RAW), the first Act square waits on all DVE
    bn_stats (clock-compression proxy), and same-engine sem self-chains add
    ~160ns per instruction.  Rewrite the tail waits to precise PE ticks:
    psum region b's last write is matmul tick 300 + b + 1 (15 full dc rounds
    of NBLK matmuls, then the final-dc matmuls in block order)."""
    insts = [i for f in nc.m.functions for bb in f.blocks for i in bb.instructions]
    matmuls = [i for i in insts if i.opcode == "Matmult"]
    n_mm = len(matmuls)
    assert n_mm == NDC * NBLK + 1, n_mm  # +1 p-state warm-up matmul
    # find the PE completion-sem (the update attached to matmuls)
    pe_upd = None
    for i in matmuls:
        si = i.sync_info
        if si and si.on_update:
            for u in si.on_update:
                if u.ant_name and u.ant_name.startswith("PE"):
                    pe_upd = u
    assert pe_upd is not None

    def pe_wait(tick):
        return mybir.SyncWait(
            sync_type="semaphore",
            id=pe_upd.id,
            ant_name=pe_upd.ant_name,
            wait_mode="sem-ge-imm",
            wait_value=tick,
        )

    base = n_mm - NBLK  # matmul ticks before the final dc round
    bn = [i for i in insts if i.opcode == "BNStats"]
    assert len(bn) == N_PAIR
    for j, i in enumerate(bn):
        bmax = N_ACT + 2 * j + 1
        i.sync_info.on_wait = [pe_wait(base + bmax + 1)]
    acts = [i for i in insts if i.opcode == "Activation"]
    assert len(acts) == N_ACT
    # first Act square: precise PE tick covering all Act blocks (strictly
    # correct for the whole chain); later ones keep the Act self-chain, which
    # also serializes the shared hw accumulator register.
    acts[0].sync_info.on_wait = [pe_wait(base + N_ACT)]


def fix_prepared_out(nc):
    """Tile routes each Pool-engine DMA prep to a DMASW semaphore lane and
    the kernel-exit drain waits for those lane sems to reach 16, but the
    completion sem baked into a prepare_only descriptor is the user-passed
    `sem=` handle.  Point each prep's completion update at its assigned
    DMASW lane sem (found from the drain's own waits, in round-robin order)
    so the hardware DMA-complete increments what the drain is waiting on."""
    insts = [i for f in nc.m.functions for bb in f.blocks for i in bb.instructions]
    lane_sems = {}
    for i in insts:
        si = i.sync_info
        for w in (si.on_wait or []) if si else []:
            if w.ant_name and w.ant_name.startswith("DMASW"):
                lane = int(w.ant_name[5 : w.ant_name.index("_")])
                lane_sems[lane] = (w.id, w.ant_name)
    preps = [i for i in insts if i.opcode == "KVWritebackAnt"]
    assert len(preps) == len(lane_sems), (len(preps), lane_sems)
    for j, i in enumerate(preps):
        sid, sname = lane_sems[j]
        for u in i.sync_info.on_update or []:
            if u.ant_name == "out_dma_sem":
                u.id = sid
                u.ant_name = sname


def build_program(sim_mode: bool = False):
    nc = bass.Bass(
        "TRN2", target_bir_lowering=False, debug=False, num_devices=N_CORES
    )
    x = nc.dram_tensor("x", [N, COLS], FP8, kind="ExternalInput").ap()
    out_c = nc.dram_tensor("out_c", [P, OUTW_A + OUTW_D], F32, kind="ExternalOutput").ap()

    with _TC(nc) as tc:
        with (
            tc.tile_pool(name="stream", bufs=1) as stream,
            tc.tile_pool(name="tail", bufs=1) as tail,
            tc.tile_pool(name="psum", bufs=1, space="PSUM") as psum,
        ):
            ps = psum.tile([P, NBLK * P], F32, name="ps", tag="ps")

            acc = tail.tile([P, OUTW_A + OUTW_D], F32, name="acc")
            acc_a = acc[:, 0:OUTW_A]
            acc_d = acc[:, OUTW_A : OUTW_A + OUTW_D]

            # PE p-state warm-up: one tiny all-zeros matmul right after the
            # preamble starts the 3us ramp clock, so the real matmuls (first
            # data lands ~3.6us) run at full 2.4 GHz from the start.  It
            # doubles as bank 0's start=True zeroing (contributes exact 0s),
            # so blocks 0..3 accumulate with start=False below.
            warm = tail.tile([P, 2, P], FP8, name="warm")
            nc.gpsimd.memset(warm, 0)
            nc.tensor.matmul(
                ps[:, 0:P], lhsT=warm, rhs=warm,
                start=True, stop=False, perf_mode=DR,
            )

            xr = x.rearrange("(c p) d -> p c d", p=P)
            off = 0
            dc = 0
            for g, m in enumerate(GROUP_SIZES):
                tg = stream.tile([P, m, COLS], FP8, name=f"s{g}", tag=f"s{g}")
                nc.sync.dma_start(out=tg, in_=xr[:, off : off + m, :])
                for jj in range(m // 2):
                    pr = slice(2 * jj, 2 * jj + 2)
                    for pos, (u, v) in enumerate(PATTERN):
                        # start=True zeroes the ENTIRE psum bank, so only the
                        # first block of each 4-block bank may carry it; the
                        # other blocks accumulate onto the freshly-zeroed bank.
                        nc.tensor.matmul(
                            ps[:, P * pos : P * (pos + 1)],
                            lhsT=tg[:, pr, P * u : P * (u + 1)],
                            rhs=tg[:, pr, P * v : P * (v + 1)],
                            start=(dc == 0 and pos % 4 == 0 and pos > 0),
                            stop=(dc == NDC - 1),
                            perf_mode=DR,
                        )
                    dc += 1
                off += m

            # ---- tail: per-block sum(A^2) partials --------------------------
            # Act: blocks 0..N_ACT-1 (their psum regions close first), square
            # activation with accumulator -> acc_a[:, b].
            # DVE: remaining blocks via parity-paired bn_stats: the input AP
            # interleaves two 128-wide blocks at element granularity (outer
            # dim stride 1, inner dim stride 128), so the even-parity stats
            # are the first block and the odd-parity stats the second; host
            # recovers sum(x^2) = M2 + n*mean^2 per parity.  Separate output
            # tiles per engine so the two reduce streams proceed
            # independently.
            scr = tail.tile([P, P], F32, name="scr")
            for p_ in range(N_PAIR):
                b0 = N_ACT + 2 * p_
                pv = ps[:, P * b0 : P * (b0 + 2)].rearrange(
                    "p (b i) -> p i b", b=2
                )
                # emit InstBNStats directly: the bass wrapper insists batched
                # outputs for multi-dim inputs, but walrus only accepts the
                # plain 6-per-partition form; the DVE streams the input AP in
                # order, so this interleaved view alternates the two blocks
                # and the even/odd parity stats separate them again.
                nc.vector.add_instruction(
                    mybir.InstBNStats(
                        name=nc.get_next_instruction_name(),
                        ins=[nc.vector.lower_ap(pv)],
                        outs=[nc.vector.lower_ap(acc_d[:, 6 * p_ : 6 * p_ + 6])],
                    )
                )
            for i in range(N_ACT):
                nc.scalar.activation(
                    out=scr,
                    in_=ps[:, P * i : P * (i + 1)],
                    func=mybir.ActivationFunctionType.Square,
                    accum_out=acc_a[:, i : i + 1],
                )
            nc.sync.dma_start(out=out_c, in_=acc)

    fix_tail_sync(nc)
    split_excess_waits(nc)
    tune_const_memsets(nc)
    return nc


_CACHE = {}


def _prep(x: np.ndarray) -> np.ndarray:
    """Row-normalize to norm SCALE and quantize to fp8e4m3."""
    import ml_dtypes

    xf = np.ascontiguousarray(np.asarray(x, dtype=np.float32))
    n = np.sqrt(np.einsum("nd,nd->n", xf, xf))
    u = xf * (SCALE / (n + 1e-7))[:, None]
    return u.astype(ml_dtypes.float8_e4m3)


def _block_norms(aa: np.ndarray, ad: np.ndarray) -> list[float]:
    """Per-block ||A_b||_F^2 from one core's out_a [P, N_ACT] (Act accums)
    and out_dv [P, 6*N_PAIR] (parity-paired bn_stats), both float64."""
    norms = [0.0] * NBLK
    for i in range(N_ACT):
        norms[i] = float(aa[:, i].sum())
    for b in range(N_ACT, NBLK):
        p_, par = (b - N_ACT) // 2, (b - N_ACT) % 2
        st = ad[:, 6 * p_ + 3 * par : 6 * p_ + 3 * par + 3]
        norms[b] = float((st[:, 2] + st[:, 0] * st[:, 1] ** 2).sum())
    return norms


def kernel(feat_q: np.ndarray, feat_k: np.ndarray) -> np.ndarray:
    assert feat_q.shape == (N, D) and feat_k.shape == (N, D)

    if "nc" not in _CACHE:
        _CACHE["nc"] = build_program()
    nc = _CACHE["nc"]

    uq8 = _prep(feat_q)
    uk8 = _prep(feat_k)
    w8 = np.concatenate([uq8, uk8], axis=1)      # [N, 2D]; chunk i = 128 cols
    in_maps = []
    for c in range(N_CORES):
        cols = np.concatenate(
            [w8[:, P * ch : P * (ch + 1)] for ch in PHIS[c]], axis=1
        )
        in_maps.append({"x": np.ascontiguousarray(cols)})
    res = run_bass_kernel_spmd(nc, in_maps, list(range(N_CORES)))

    vals = {}
    for c in range(N_CORES):
        ac = np.asarray(res.results[c]["out_c"], dtype=np.float64)
        aa, ad = ac[:, :OUTW_A], ac[:, OUTW_A:]
        norms = _block_norms(aa, ad)
        phi = PHIS[c]
        for pos, (u, v) in enumerate(PATTERN):
            x_, y_ = phi[u], phi[v]
            key = (min(x_, y_), max(x_, y_))
            if key not in vals:
                vals[key] = norms[pos]

    nq = D // P   # 8 chunks per tensor
    total = 0.0
    for i in range(2 * nq):
        for j in range(i, 2 * nq):
            v = vals[(i, j)]
            if (i < nq) == (j < nq):
                total += v if i == j else 2.0 * v
            else:
                total += -2.0 * v
    loss = total / (SCALE**4) / (N * (N - 1))
    return np.asarray(loss, dtype=np.float32)


if __name__ == "__main__":
    rng = np.random.default_rng(0)
    q = rng.standard_normal((N, D)).astype(np.float32)
    k = rng.standard_normal((N, D)).astype(np.float32)
    print("loss:", kernel(q, k))
